# revision 7
# baseline (speedup 1.0000x reference)
"""ChildSumTreeLSTM on 8 trn2 NeuronCores — v2 (fused).

Tree: reversed complete 4-ary heap (id = N-1-heap; heap j's children are
4j+1..4j+4).  The 64 depth-3 subtrees rooted at heap 21..84 are dealt to
cores stride-8 (subtree k -> core k%8, slot k//8) so that every core's REAL
level-6 leaves fit in its first 6 subtree slots (384 leaf columns); the
last 2 slots are always leafless, so the leaf level computes 384 columns
instead of 512.  Each core runs a uniform padded forest (levels 384-of-512
/128/32/8) and then every core redundantly computes the 21-node top tree
after a 32KB AllGather of the 64 subtree roots.

Layouts ("T layout"): mem (512 -> 4 partition chunks of 128) on partitions,
node slots on the free dim.  Per-level state H/C are mono-tiles
[128, 4*slots] (m-major columns) so each gate stage is ONE wide instruction
with nested access patterns instead of 4-12 small ones.  Leaf gates are
computed by the scalar engine directly from PSUM (bias bx+bs fused into the
activation), so leaf X projections are never materialized in SBUF.
"""

import os
import sys

sys.path.insert(0, "/opt/trn_rl_repo")

import numpy as np

import concourse.bass as bass
import concourse.bacc as bacc
import concourse.mybir as mybir
import concourse.tile as tile
from concourse.bass_utils import run_bass_kernel_spmd

F32 = mybir.dt.float32
F16 = mybir.dt.float16  # GEMM operand dtype (single-pass PE, 10-bit mantissa)
AF = mybir.ActivationFunctionType
ALU = mybir.AluOpType
AX = mybir.AxisListType

N = 4096
MEM = 512
IN_DIM = 512
NCORES = 8
P = 128
KT = 4  # contraction tiles (512 / 128)

# per-core column layout: internal+top region then compacted leaf region
OFF2, OFF1, OFF0 = 0, 128, 160
OFFT2, OFFT1, OFFT0 = 168, 184, 188
NI = 192                 # internal + top cols (3 pad at 189..191)
NLF = 384                # computed leaf cols (6 subtrees x 64)
NX = NI + NLF            # xin cols
NL3, NL2, NL1, NL0 = 512, 128, 32, 8

LAST_RESULT = None  # BassKernelResults of the most recent run (for test.py)


def _core_heaps(c):
    heaps = np.full(NX, -1, dtype=np.int64)
    for s in range(8):
        t = 21 + 8 * s + c
        for a in range(16):
            heaps[OFF2 + 16 * s + a] = 16 * t + 5 + a
        for b in range(4):
            heaps[OFF1 + 4 * s + b] = 4 * t + 1 + b
        heaps[OFF0 + s] = t
    heaps[OFFT2:OFFT2 + 16] = np.arange(5, 21)
    heaps[OFFT1:OFFT1 + 4] = np.arange(1, 5)
    heaps[OFFT0] = 0
    for s in range(6):
        t = 21 + 8 * s + c
        for e in range(64):
            h = 64 * t + 21 + e
            heaps[NI + 64 * s + e] = h if h < N else -1
    return heaps


def _bcast4(ap, n):
    """broadcast the innermost dim 4x: [P, n] -> [P, n, 4(stride 0)]"""
    return bass.AP(tensor=ap.tensor, offset=ap.offset,
                   ap=list(ap.ap) + [[0, 4]])


def _build_program():
    nc = bacc.Bacc("TRN2", target_bir_lowering=False, debug=False)

    xin_d = nc.dram_tensor("xin", [IN_DIM, NX], F16, kind="ExternalInput")
    wx_d = nc.dram_tensor("wx", [IN_DIM, 4 * MEM], F16, kind="ExternalInput")
    ws_d = nc.dram_tensor("ws", [MEM, 3 * MEM], F16, kind="ExternalInput")
    wf_d = nc.dram_tensor("wf", [MEM, MEM], F16, kind="ExternalInput")
    bxc_d = nc.dram_tensor("bxc", [P, 16], F32, kind="ExternalInput")  # bx (+bs on iou blocks)
    bxs_d = nc.dram_tensor("bxs", [P, 12], F32, kind="ExternalInput")  # bx+bs for leaf i,o,u
    bf_d = nc.dram_tensor("bf", [P, 4], F32, kind="ExternalInput")
    cm_d = nc.dram_tensor("cmask", [P, NLF], F32, kind="ExternalInput")
    out_d = nc.dram_tensor("out", [P, KT], F32, kind="ExternalOutput")
    contrib_d = nc.dram_tensor("contrib", [2 * MEM, NL0], F32)
    gath_d = nc.dram_tensor("gath", [NCORES * 2 * MEM, NL0], F32,
                            addr_space="Shared")

    with tile.TileContext(nc) as tc:
        with (
            tc.tile_pool(name="wpool", bufs=1) as wpool,
            tc.tile_pool(name="state", bufs=1) as state,
            tc.tile_pool(name="tmp", bufs=1) as tmp,
            tc.tile_pool(name="ps", bufs=1, space="PSUM") as ps,
        ):
            # ---- load everything (wx+xin first: phase A starts on them) ----
            wx_s = [wpool.tile([P, 4 * MEM], F16, name="t", tag=f"wx{k}") for k in range(KT)]
            ws_s = [wpool.tile([P, 3 * MEM], F16, name="t", tag=f"ws{k}") for k in range(KT)]
            wf_s = [wpool.tile([P, MEM], F16, name="t", tag=f"wf{k}") for k in range(KT)]
            in_s = [wpool.tile([P, NX], F16, name="t", tag=f"in{k}") for k in range(KT)]
            for k in range(KT):
                r = slice(k * P, (k + 1) * P)
                nc.sync.dma_start(wx_s[k][:], wx_d[r, :])
                nc.sync.dma_start(in_s[k][:], xin_d[r, :])
            bxc_s = wpool.tile([P, 16], F32, name="t", tag="bxc")
            bxs_s = wpool.tile([P, 12], F32, name="t", tag="bxs")
            bf_s = wpool.tile([P, 4], F32, name="t", tag="bf")
            cm_s = wpool.tile([P, NLF], F32, name="t", tag="cm")
            nc.sync.dma_start(bxs_s[:], bxs_d[:])
            nc.sync.dma_start(bxc_s[:], bxc_d[:])
            nc.sync.dma_start(bf_s[:], bf_d[:])
            nc.sync.dma_start(cm_s[:], cm_d[:])
            for k in range(KT):
                r = slice(k * P, (k + 1) * P)
                nc.sync.dma_start(ws_s[k][:], ws_d[r, :])
                nc.sync.dma_start(wf_s[k][:], wf_d[r, :])

            # ---- phase A (leaf): gates straight from PSUM, bias fused ----
            # 12 GEMMs [128, NLF]; sigma/tanh reads psum, writes mono gate tile
            IG = tmp.tile([P, KT * NLF], F32, name="t", tag="IG", bufs=1)
            OG = tmp.tile([P, KT * NLF], F32, name="t", tag="OG", bufs=1)
            UG = tmp.tile([P, KT * NLF], F32, name="t", tag="UG", bufs=1)
            leaf_jobs = []
            for m in range(KT):
                leaf_jobs += [(m, IG, AF.Sigmoid, m), (12 + m, UG, AF.Tanh, 8 + m),
                              (8 + m, OG, AF.Sigmoid, 4 + m)]
            for mc, gt, fn, bcol in leaf_jobs:
                m = mc % 4
                p_l = ps.tile([P, NLF], F32, name="t", tag="psA", bufs=3)
                for k in range(KT):
                    nc.tensor.matmul(
                        p_l[:], wx_s[k][:, mc * P:(mc + 1) * P],
                        in_s[k][:, NI:NX],
                        start=(k == 0), stop=(k == KT - 1),
                    )
                nc.scalar.activation(gt[:, m * NLF:(m + 1) * NLF], p_l[:],
                                     fn, bias=bxs_s[:, bcol:bcol + 1])

            # ---- phase A (internal): Xt mono [128, 16*NI], bias bxc fused ----
            Xt = tmp.tile([P, 16 * NI], F32, name="t", tag="Xt", bufs=1)
            for i, mc in enumerate([4, 5, 6, 7] + [0, 1, 2, 3] + list(range(8, 16))):
                p_i = ps.tile([P, NLF], F32, name="t", tag="psA", bufs=3)
                for k in range(KT):
                    nc.tensor.matmul(
                        p_i[:, :NI], wx_s[k][:, mc * P:(mc + 1) * P],
                        in_s[k][:, 0:NI],
                        start=(k == 0), stop=(k == KT - 1),
                    )
                if i % 2 == 0:
                    nc.vector.tensor_scalar_add(Xt[:, mc * NI:(mc + 1) * NI],
                                                p_i[:, :NI], bxc_s[:, mc:mc + 1])
                else:
                    nc.scalar.activation(Xt[:, mc * NI:(mc + 1) * NI],
                                         p_i[:, :NI], AF.Identity,
                                         bias=bxc_s[:, mc:mc + 1])

            # ---- leaf c/h into mono state H3/C3 [128, 4*512] ----
            H3 = state.tile([P, KT * NL3], F16, name="t", tag="H3")
            C3 = state.tile([P, KT * NL3], F32, name="t", tag="C3")
            # pad slots 384..511 of each m-chunk are zero
            padap = lambda t: bass.AP(tensor=t.tensor, offset=t.offset + NLF,
                                      ap=[t.ap[0], [NL3, KT], [1, NL3 - NLF]])
            nc.gpsimd.memset(padap(H3[:]), 0.0)
            nc.gpsimd.memset(padap(C3[:]), 0.0)
            CR = tmp.tile([P, KT * NLF], F32, name="t", tag="CR", bufs=1)
            nc.vector.tensor_mul(CR[:], IG[:], UG[:])
            # C3[:, m*512 + 0:384] = CR * cmask (mask broadcast over m)
            c3l = lambda t: bass.AP(tensor=t.tensor, offset=t.offset,
                                    ap=[t.ap[0], [NL3, KT], [1, NLF]])
            cmb = bass.AP(tensor=cm_s.tensor, offset=cm_s.offset,
                          ap=[cm_s.ap[0], [0, KT], [1, NLF]])
            crv = CR[:].rearrange("p (m e) -> p m e", m=KT)
            nc.gpsimd.tensor_mul(c3l(C3[:]), crv, cmb)
            THL = tmp.tile([P, KT * NLF], F32, name="t", tag="THL", bufs=1)
            nc.scalar.activation(THL[:].rearrange("p (m e) -> p m e", m=KT),
                                 c3l(C3[:]), AF.Tanh)
            nc.vector.tensor_mul(c3l(H3[:]),
                                 OG[:].rearrange("p (m e) -> p m e", m=KT),
                                 THL[:].rearrange("p (m e) -> p m e", m=KT))

            def level_step(n_par, x_off, Hc, Cc, hname, h_dtype=F16):
                """One fused ChildSumTreeLSTM level in T layout.
                Hc/Cc: mono child tiles [128, 4*nch]; returns mono Hp/Cp."""
                nch = 4 * n_par
                # f = sigmoid(Wf.T @ Hc + fx + bf); FCCS = sum4(f * Cc)
                F = tmp.tile([P, KT * nch], F32, name="t", tag="F")
                for m in range(KT):
                    p_f = ps.tile([P, MEM], F32, name="t", tag="psF", bufs=2)
                    for k in range(KT):
                        nc.tensor.matmul(
                            p_f[:, :nch], wf_s[k][:, m * P:(m + 1) * P],
                            Hc[:, k * nch:(k + 1) * nch],
                            start=(k == 0), stop=(k == KT - 1),
                        )
                    fx = bass.AP(tensor=Xt.tensor,
                                 offset=Xt.offset + (4 + m) * NI + x_off,
                                 ap=[Xt.ap[0], [1, n_par], [0, 4]])
                    tf = tmp.tile([P, nch], F32, name="t", tag="tf", bufs=2)
                    nc.vector.tensor_add(tf[:].rearrange("p (n g) -> p n g", g=4),
                                   p_f[:, :nch].rearrange("p (n g) -> p n g", g=4),
                                   fx)
                    nc.scalar.activation(F[:, m * nch:(m + 1) * nch], tf[:],
                                         AF.Sigmoid, bias=bf_s[:, m:m + 1])
                FCC = tmp.tile([P, KT * nch], F16, name="t", tag="FCC")
                nc.gpsimd.tensor_mul(FCC[:], F[:], Cc[:])
                FCCS = tmp.tile([P, KT * n_par], F32, name="t", tag="FS")
                nc.vector.tensor_reduce(
                    FCCS[:].rearrange("p (m n) -> p m n", m=KT),
                    FCC[:].rearrange("p (m n g) -> p m n g", m=KT, g=4),
                    axis=AX.X, op=ALU.add,
                )
                # child-h sum (groups of 4 adjacent columns), f16 accum is
                # fine: 4-term sums of f16 h values
                CHS = tmp.tile([P, KT * n_par], F16, name="t", tag="CH")
                with nc.allow_low_precision("4-term child-h sum in f16"):
                    nc.vector.tensor_reduce(
                        CHS[:].rearrange("p (k n) -> p k n", k=KT),
                        Hc[:].rearrange("p (k n g) -> p k n g", k=KT, g=4),
                        axis=AX.X, op=ALU.add,
                    )
                # iou = Ws.T @ chs into one mono psum [128, 12*n_par]
                p_b = ps.tile([P, 12 * P], F32, name="t", tag="psB", bufs=1)
                for mc in range(12):
                    for k in range(KT):
                        nc.tensor.matmul(
                            p_b[:, mc * n_par:(mc + 1) * n_par],
                            ws_s[k][:, mc * P:(mc + 1) * P],
                            CHS[:, k * n_par:(k + 1) * n_par],
                            start=(k == 0), stop=(k == KT - 1),
                        )
                # gates: PG = psum + Xt (bs already folded into Xt's iou cols)
                PG = tmp.tile([P, 12 * n_par], F32, name="t", tag="PG")
                xi = bass.AP(tensor=Xt.tensor, offset=Xt.offset + x_off,
                             ap=[Xt.ap[0], [NI, 4], [1, n_par]])
                xou = bass.AP(tensor=Xt.tensor,
                              offset=Xt.offset + 8 * NI + x_off,
                              ap=[Xt.ap[0], [NI, 8], [1, n_par]])
                nc.vector.tensor_add(
                    PG[:, :4 * n_par].rearrange("p (m n) -> p m n", m=4),
                    p_b[:, :4 * n_par].rearrange("p (m n) -> p m n", m=4), xi)
                nc.vector.tensor_add(
                    PG[:, 4 * n_par:].rearrange("p (m n) -> p m n", m=8),
                    p_b[:, 4 * n_par:12 * n_par].rearrange("p (m n) -> p m n", m=8),
                    xou)
                GG = tmp.tile([P, 12 * n_par], F32, name="t", tag="GG")
                nc.scalar.activation(GG[:, :8 * n_par], PG[:, :8 * n_par],
                                     AF.Sigmoid)
                nc.scalar.activation(GG[:, 8 * n_par:], PG[:, 8 * n_par:],
                                     AF.Tanh)
                IU = tmp.tile([P, KT * n_par], F32, name="t", tag="IU")
                nc.gpsimd.tensor_mul(IU[:], GG[:, :4 * n_par],
                                     GG[:, 8 * n_par:12 * n_par])
                Cp = state.tile([P, KT * n_par], F32, name="t", tag=f"C{hname}")
                nc.gpsimd.tensor_add(Cp[:], IU[:], FCCS[:])
                TH = tmp.tile([P, KT * n_par], F32, name="t", tag="TH")
                nc.scalar.activation(TH[:], Cp[:], AF.Tanh)
                Hp = state.tile([P, KT * n_par], h_dtype, name="t", tag=f"H{hname}")
                nc.gpsimd.tensor_mul(Hp[:], GG[:, 4 * n_par:8 * n_par], TH[:])
                return Hp, Cp

            H2, C2 = level_step(NL2, OFF2, H3, C3, "L2")
            H1, C1 = level_step(NL1, OFF1, H2, C2, "L1")
            H0, C0 = level_step(NL0, OFF0, H1, C1, "L0")

            # ---- gather the 64 subtree roots (h and c) to every core ----
            h0f = tmp.tile([P, KT * NL0], F32, name="t", tag="h0f", bufs=1)
            nc.gpsimd.tensor_copy(h0f[:], H0[:])
            for m in range(KT):
                nc.sync.dma_start(contrib_d[m * P:(m + 1) * P, :],
                                  h0f[:, m * NL0:(m + 1) * NL0])
                nc.sync.dma_start(contrib_d[MEM + m * P:MEM + (m + 1) * P, :],
                                  C0[:, m * NL0:(m + 1) * NL0])
            nc.gpsimd.collective_compute(
                "AllGather", ALU.bypass,
                replica_groups=[list(range(NCORES))],
                ins=[contrib_d[:]],
                outs=[gath_d[:]],
            )
            # gath rows: 1024*c + 512*hc + 128*m + p ; cols: slot s.
            # subtree k = 8*s + c  ->  T2 child column k  (s outer, c inner)
            H64f = state.tile([P, KT * 64], F32, name="t", tag="H64f")
            H64 = state.tile([P, KT * 64], F16, name="t", tag="H64")
            C64 = state.tile([P, KT * 64], F32, name="t", tag="C64")
            gv = gath_d[:].rearrange("(c hc m p) s -> hc m p s c",
                                     c=NCORES, hc=2, m=KT)
            for m in range(KT):
                nc.sync.dma_start(
                    H64f[:, m * 64:(m + 1) * 64].rearrange(
                        "p (s c) -> p s c", s=NL0), gv[0, m])
                nc.sync.dma_start(
                    C64[:, m * 64:(m + 1) * 64].rearrange(
                        "p (s c) -> p s c", s=NL0), gv[1, m])
            nc.gpsimd.tensor_copy(H64[:], H64f[:])

            HT2, CT2 = level_step(16, OFFT2, H64, C64, "T2")
            HT1, CT1 = level_step(4, OFFT1, HT2, CT2, "T1")
            HT0, _ = level_step(1, OFFT0, HT1, CT1, "T0", h_dtype=F32)
            nc.sync.dma_start(out_d[:], HT0[:])

    nc.compile()
    return nc


_NC_CACHE = None


def kernel(inputs, Wx, bx, Ws, bs, Wf, bf, children):
    global LAST_RESULT, _NC_CACHE
    inputs = np.asarray(inputs, np.float32)
    Wx = np.asarray(Wx, np.float32)
    bx = np.asarray(bx, np.float32)
    Ws = np.asarray(Ws, np.float32)
    bs = np.asarray(bs, np.float32)
    Wf = np.asarray(Wf, np.float32)
    bf = np.asarray(bf, np.float32)

    Wx_b = Wx.astype(np.float16)
    Ws_b = Ws.astype(np.float16)
    Wf_b = Wf.astype(np.float16)
    bxT = bx.reshape(16, P).T          # [128, 16] col mc
    bsT = bs.reshape(12, P).T          # [128, 12] col: i0..3 o0..3 u0..3
    bfT = np.ascontiguousarray(bf.reshape(4, P).T)
    # bxc: bx with bs folded into the i/o/u blocks (internal-X bias)
    bxc = bxT.copy()
    bxc[:, 0:4] += bsT[:, 0:4]
    bxc[:, 8:12] += bsT[:, 4:8]
    bxc[:, 12:16] += bsT[:, 8:12]
    bxc = np.ascontiguousarray(bxc)
    # bxs: leaf-gate bias (i,o,u blocks of bx + bs)
    bxs = np.concatenate(
        [bxT[:, 0:4] + bsT[:, 0:4], bxT[:, 8:12] + bsT[:, 4:8],
         bxT[:, 12:16] + bsT[:, 8:12]], axis=1)
    bxs = np.ascontiguousarray(bxs)

    in_maps = []
    for c in range(NCORES):
        heaps = _core_heaps(c)
        valid = heaps >= 0
        M = np.zeros((NX, IN_DIM), np.float32)
        M[valid] = inputs[N - 1 - heaps[valid]]
        xin = np.ascontiguousarray(M.T)
        mrow = valid[NI:].astype(np.float32)
        cmask = np.ascontiguousarray(np.tile(mrow[None, :], (P, 1)))
        in_maps.append({
            "xin": xin.astype(np.float16), "wx": Wx_b, "ws": Ws_b,
            "wf": Wf_b, "bxc": bxc, "bxs": bxs, "bf": bfT, "cmask": cmask,
        })

    if _NC_CACHE is None:
        _NC_CACHE = _build_program()
    nc = _NC_CACHE

    res = run_bass_kernel_spmd(
        nc, in_maps, list(range(NCORES)),
        trace=bool(os.environ.get("BASS_TRACE")),
    )
    LAST_RESULT = res

    out = np.asarray(res.results[0]["out"])  # [128, 4]; h[m*128+p] = out[p, m]
    return np.ascontiguousarray(out.T.reshape(1, MEM))


# revision 8
# speedup vs baseline: 1.6441x; 1.6441x over previous
"""ChildSumTreeLSTM on 8 trn2 NeuronCores — v2 (fused).

Tree: reversed complete 4-ary heap (id = N-1-heap; heap j's children are
4j+1..4j+4).  The 64 depth-3 subtrees rooted at heap 21..84 are dealt to
cores stride-8 (subtree k -> core k%8, slot k//8) so that every core's REAL
level-6 leaves fit in its first 6 subtree slots (384 leaf columns); the
last 2 slots are always leafless, so the leaf level computes 384 columns
instead of 512.  Each core runs a uniform padded forest (levels 384-of-512
/128/32/8) and then every core redundantly computes the 21-node top tree
after a 32KB AllGather of the 64 subtree roots.

Layouts ("T layout"): mem (512 -> 4 partition chunks of 128) on partitions,
node slots on the free dim.  Per-level state H/C are mono-tiles
[128, 4*slots] (m-major columns) so each gate stage is ONE wide instruction
with nested access patterns instead of 4-12 small ones.  Leaf gates are
computed by the scalar engine directly from PSUM (bias bx+bs fused into the
activation), so leaf X projections are never materialized in SBUF.
"""

import os
import sys

sys.path.insert(0, "/opt/trn_rl_repo")

import numpy as np

import concourse.bass as bass
import concourse.bacc as bacc
import concourse.mybir as mybir
import concourse.tile as tile
from concourse.bass_utils import run_bass_kernel_spmd

F32 = mybir.dt.float32
F16 = mybir.dt.float16  # GEMM operand dtype (single-pass PE, 10-bit mantissa)
AF = mybir.ActivationFunctionType
ALU = mybir.AluOpType
AX = mybir.AxisListType

N = 4096
MEM = 512
IN_DIM = 512
NCORES = 8
P = 128
KT = 4  # contraction tiles (512 / 128)

# per-core column layout: internal+top region then compacted leaf region
OFF2, OFF1, OFF0 = 0, 128, 160
OFFT2, OFFT1, OFFT0 = 168, 184, 188
NI = 192                 # internal + top cols (3 pad at 189..191)
NLF = 384                # computed leaf cols (6 subtrees x 64)
NX = NI + NLF            # xin cols
NL3, NL2, NL1, NL0 = 512, 128, 32, 8

LAST_RESULT = None  # BassKernelResults of the most recent run (for test.py)


def _core_heaps(c):
    heaps = np.full(NX, -1, dtype=np.int64)
    for s in range(8):
        t = 21 + 8 * s + c
        for a in range(16):
            heaps[OFF2 + 16 * s + a] = 16 * t + 5 + a
        for b in range(4):
            heaps[OFF1 + 4 * s + b] = 4 * t + 1 + b
        heaps[OFF0 + s] = t
    heaps[OFFT2:OFFT2 + 16] = np.arange(5, 21)
    heaps[OFFT1:OFFT1 + 4] = np.arange(1, 5)
    heaps[OFFT0] = 0
    for s in range(6):
        t = 21 + 8 * s + c
        for e in range(64):
            h = 64 * t + 21 + e
            heaps[NI + 64 * s + e] = h if h < N else -1
    return heaps


def _bcast4(ap, n):
    """broadcast the innermost dim 4x: [P, n] -> [P, n, 4(stride 0)]"""
    return bass.AP(tensor=ap.tensor, offset=ap.offset,
                   ap=list(ap.ap) + [[0, 4]])


def _build_program():
    nc = bacc.Bacc("TRN2", target_bir_lowering=False, debug=False)

    xin_d = nc.dram_tensor("xin", [IN_DIM, NX], F16, kind="ExternalInput")
    wx_d = nc.dram_tensor("wx", [IN_DIM, 4 * MEM], F16, kind="ExternalInput")
    ws_d = nc.dram_tensor("ws", [MEM, 3 * MEM], F16, kind="ExternalInput")
    wf_d = nc.dram_tensor("wf", [MEM, MEM], F16, kind="ExternalInput")
    bxc_d = nc.dram_tensor("bxc", [P, 16], F32, kind="ExternalInput")  # bx (+bs on iou blocks)
    bxs_d = nc.dram_tensor("bxs", [P, 12], F32, kind="ExternalInput")  # bx+bs for leaf i,o,u
    bf_d = nc.dram_tensor("bf", [P, 4], F32, kind="ExternalInput")
    cm_d = nc.dram_tensor("cmask", [P, NLF], F32, kind="ExternalInput")
    out_d = nc.dram_tensor("out", [P, KT], F32, kind="ExternalOutput")
    contrib_d = nc.dram_tensor("contrib", [2 * MEM, NL0], F32)
    gath_d = nc.dram_tensor("gath", [NCORES * 2 * MEM, NL0], F32,
                            addr_space="Shared")

    with tile.TileContext(nc) as tc:
        with (
            tc.tile_pool(name="wpool", bufs=1) as wpool,
            tc.tile_pool(name="state", bufs=1) as state,
            tc.tile_pool(name="tmp", bufs=1) as tmp,
            tc.tile_pool(name="ps", bufs=1, space="PSUM") as ps,
        ):
            # ---- load everything (wx+xin first: phase A starts on them) ----
            wx_s = [wpool.tile([P, 4 * MEM], F16, name="t", tag=f"wx{k}") for k in range(KT)]
            ws_s = [wpool.tile([P, 3 * MEM], F16, name="t", tag=f"ws{k}") for k in range(KT)]
            wf_s = [wpool.tile([P, MEM], F16, name="t", tag=f"wf{k}") for k in range(KT)]
            in_s = [wpool.tile([P, NX], F16, name="t", tag=f"in{k}") for k in range(KT)]
            for k in range(KT):
                r = slice(k * P, (k + 1) * P)
                nc.sync.dma_start(wx_s[k][:], wx_d[r, :])
                nc.sync.dma_start(in_s[k][:], xin_d[r, :])
            bxc_s = wpool.tile([P, 16], F32, name="t", tag="bxc")
            bxs_s = wpool.tile([P, 12], F32, name="t", tag="bxs")
            bf_s = wpool.tile([P, 4], F32, name="t", tag="bf")
            cm_s = wpool.tile([P, NLF], F32, name="t", tag="cm")
            nc.sync.dma_start(bxs_s[:], bxs_d[:])
            nc.sync.dma_start(bxc_s[:], bxc_d[:])
            nc.sync.dma_start(bf_s[:], bf_d[:])
            nc.sync.dma_start(cm_s[:], cm_d[:])
            for k in range(KT):
                r = slice(k * P, (k + 1) * P)
                nc.sync.dma_start(ws_s[k][:], ws_d[r, :])
                nc.sync.dma_start(wf_s[k][:], wf_d[r, :])

            # ---- phase A (leaf): gates straight from PSUM, bias fused ----
            # 12 GEMMs [128, NLF]; sigma/tanh reads psum, writes mono gate tile
            IG = tmp.tile([P, KT * NLF], F32, name="t", tag="IG", bufs=1)
            OG = tmp.tile([P, KT * NLF], F32, name="t", tag="OG", bufs=1)
            UG = tmp.tile([P, KT * NLF], F32, name="t", tag="UG", bufs=1)
            leaf_jobs = []
            for m in range(KT):
                leaf_jobs += [(m, IG, AF.Sigmoid, m), (12 + m, UG, AF.Tanh, 8 + m),
                              (8 + m, OG, AF.Sigmoid, 4 + m)]
            for mc, gt, fn, bcol in leaf_jobs:
                m = mc % 4
                p_l = ps.tile([P, NLF], F32, name="t", tag="psA", bufs=3)
                for k in range(KT):
                    nc.tensor.matmul(
                        p_l[:], wx_s[k][:, mc * P:(mc + 1) * P],
                        in_s[k][:, NI:NX],
                        start=(k == 0), stop=(k == KT - 1),
                    )
                nc.scalar.activation(gt[:, m * NLF:(m + 1) * NLF], p_l[:],
                                     fn, bias=bxs_s[:, bcol:bcol + 1])

            # ---- phase A (internal): Xt mono [128, 16*NI], bias bxc fused ----
            Xt = tmp.tile([P, 16 * NI], F32, name="t", tag="Xt", bufs=1)
            for i, mc in enumerate([4, 5, 6, 7] + [0, 1, 2, 3] + list(range(8, 16))):
                p_i = ps.tile([P, NLF], F32, name="t", tag="psA", bufs=3)
                for k in range(KT):
                    nc.tensor.matmul(
                        p_i[:, :NI], wx_s[k][:, mc * P:(mc + 1) * P],
                        in_s[k][:, 0:NI],
                        start=(k == 0), stop=(k == KT - 1),
                    )
                if i % 2 == 0:
                    nc.vector.tensor_scalar_add(Xt[:, mc * NI:(mc + 1) * NI],
                                                p_i[:, :NI], bxc_s[:, mc:mc + 1])
                else:
                    nc.scalar.activation(Xt[:, mc * NI:(mc + 1) * NI],
                                         p_i[:, :NI], AF.Identity,
                                         bias=bxc_s[:, mc:mc + 1])

            # ---- leaf c/h into mono state H3/C3 [128, 4*512] ----
            H3 = state.tile([P, KT * NL3], F16, name="t", tag="H3")
            C3 = state.tile([P, KT * NL3], F32, name="t", tag="C3")
            # pad slots 384..511 of each m-chunk are zero
            padap = lambda t: bass.AP(tensor=t.tensor, offset=t.offset + NLF,
                                      ap=[t.ap[0], [NL3, KT], [1, NL3 - NLF]])
            nc.gpsimd.memset(padap(H3[:]), 0.0)
            nc.gpsimd.memset(padap(C3[:]), 0.0)
            CR = tmp.tile([P, KT * NLF], F32, name="t", tag="CR", bufs=1)
            nc.vector.tensor_mul(CR[:], IG[:], UG[:])
            # C3[:, m*512 + 0:384] = CR * cmask (mask broadcast over m)
            c3l = lambda t: bass.AP(tensor=t.tensor, offset=t.offset,
                                    ap=[t.ap[0], [NL3, KT], [1, NLF]])
            cmb = bass.AP(tensor=cm_s.tensor, offset=cm_s.offset,
                          ap=[cm_s.ap[0], [0, KT], [1, NLF]])
            crv = CR[:].rearrange("p (m e) -> p m e", m=KT)
            nc.gpsimd.tensor_mul(c3l(C3[:]), crv, cmb)
            THL = tmp.tile([P, KT * NLF], F32, name="t", tag="THL", bufs=1)
            nc.scalar.activation(THL[:].rearrange("p (m e) -> p m e", m=KT),
                                 c3l(C3[:]), AF.Tanh)
            nc.vector.tensor_mul(c3l(H3[:]),
                                 OG[:].rearrange("p (m e) -> p m e", m=KT),
                                 THL[:].rearrange("p (m e) -> p m e", m=KT))

            def level_step(n_par, x_off, Hc, Cc, hname, h_dtype=F16):
                """One fused ChildSumTreeLSTM level in T layout.
                Hc/Cc: mono child tiles [128, 4*nch]; returns mono Hp/Cp."""
                nch = 4 * n_par
                # f = sigmoid(Wf.T @ Hc + fx + bf); FCCS = sum4(f * Cc)
                F = tmp.tile([P, KT * nch], F16, name="t", tag="F")
                for m in range(KT):
                    p_f = ps.tile([P, MEM], F32, name="t", tag="psF", bufs=2)
                    for k in range(KT):
                        nc.tensor.matmul(
                            p_f[:, :nch], wf_s[k][:, m * P:(m + 1) * P],
                            Hc[:, k * nch:(k + 1) * nch],
                            start=(k == 0), stop=(k == KT - 1),
                        )
                    fx = bass.AP(tensor=Xt.tensor,
                                 offset=Xt.offset + (4 + m) * NI + x_off,
                                 ap=[Xt.ap[0], [1, n_par], [0, 4]])
                    tf = tmp.tile([P, nch], F32, name="t", tag="tf", bufs=2)
                    nc.vector.tensor_add(tf[:].rearrange("p (n g) -> p n g", g=4),
                                   p_f[:, :nch].rearrange("p (n g) -> p n g", g=4),
                                   fx)
                    nc.scalar.activation(F[:, m * nch:(m + 1) * nch], tf[:],
                                         AF.Sigmoid, bias=bf_s[:, m:m + 1])
                FCC = tmp.tile([P, KT * nch], F16, name="t", tag="FCC")
                nc.vector.tensor_mul(FCC[:], F[:], Cc[:])
                FCCS = tmp.tile([P, KT * n_par], F32, name="t", tag="FS")
                nc.vector.tensor_reduce(
                    FCCS[:].rearrange("p (m n) -> p m n", m=KT),
                    FCC[:].rearrange("p (m n g) -> p m n g", m=KT, g=4),
                    axis=AX.X, op=ALU.add,
                )
                # child-h sum (groups of 4 adjacent columns), f16 accum is
                # fine: 4-term sums of f16 h values
                CHS = tmp.tile([P, KT * n_par], F16, name="t", tag="CH")
                with nc.allow_low_precision("4-term child-h sum in f16"):
                    nc.vector.tensor_reduce(
                        CHS[:].rearrange("p (k n) -> p k n", k=KT),
                        Hc[:].rearrange("p (k n g) -> p k n g", k=KT, g=4),
                        axis=AX.X, op=ALU.add,
                    )
                # iou = Ws.T @ chs into one mono psum [128, 12*n_par]
                p_b = ps.tile([P, 12 * P], F32, name="t", tag="psB", bufs=1)
                for mc in range(12):
                    for k in range(KT):
                        nc.tensor.matmul(
                            p_b[:, mc * n_par:(mc + 1) * n_par],
                            ws_s[k][:, mc * P:(mc + 1) * P],
                            CHS[:, k * n_par:(k + 1) * n_par],
                            start=(k == 0), stop=(k == KT - 1),
                        )
                # gates: PG = psum + Xt (bs already folded into Xt's iou cols)
                PG = tmp.tile([P, 12 * n_par], F32, name="t", tag="PG")
                xi = bass.AP(tensor=Xt.tensor, offset=Xt.offset + x_off,
                             ap=[Xt.ap[0], [NI, 4], [1, n_par]])
                xou = bass.AP(tensor=Xt.tensor,
                              offset=Xt.offset + 8 * NI + x_off,
                              ap=[Xt.ap[0], [NI, 8], [1, n_par]])
                nc.vector.tensor_add(
                    PG[:, :4 * n_par].rearrange("p (m n) -> p m n", m=4),
                    p_b[:, :4 * n_par].rearrange("p (m n) -> p m n", m=4), xi)
                nc.vector.tensor_add(
                    PG[:, 4 * n_par:].rearrange("p (m n) -> p m n", m=8),
                    p_b[:, 4 * n_par:12 * n_par].rearrange("p (m n) -> p m n", m=8),
                    xou)
                GG = tmp.tile([P, 12 * n_par], F32, name="t", tag="GG")
                nc.scalar.activation(GG[:, :8 * n_par], PG[:, :8 * n_par],
                                     AF.Sigmoid)
                nc.scalar.activation(GG[:, 8 * n_par:], PG[:, 8 * n_par:],
                                     AF.Tanh)
                IU = tmp.tile([P, KT * n_par], F32, name="t", tag="IU")
                nc.gpsimd.tensor_mul(IU[:], GG[:, :4 * n_par],
                                     GG[:, 8 * n_par:12 * n_par])
                Cp = state.tile([P, KT * n_par], F32, name="t", tag=f"C{hname}")
                nc.gpsimd.tensor_add(Cp[:], IU[:], FCCS[:])
                TH = tmp.tile([P, KT * n_par], F32, name="t", tag="TH")
                nc.scalar.activation(TH[:], Cp[:], AF.Tanh)
                Hp = state.tile([P, KT * n_par], h_dtype, name="t", tag=f"H{hname}")
                nc.gpsimd.tensor_mul(Hp[:], GG[:, 4 * n_par:8 * n_par], TH[:])
                return Hp, Cp

            H2, C2 = level_step(NL2, OFF2, H3, C3, "L2")
            H1, C1 = level_step(NL1, OFF1, H2, C2, "L1")
            H0, C0 = level_step(NL0, OFF0, H1, C1, "L0")

            # ---- gather the 64 subtree roots (h and c) to every core ----
            h0f = tmp.tile([P, KT * NL0], F32, name="t", tag="h0f", bufs=1)
            nc.gpsimd.tensor_copy(h0f[:], H0[:])
            for m in range(KT):
                nc.sync.dma_start(contrib_d[m * P:(m + 1) * P, :],
                                  h0f[:, m * NL0:(m + 1) * NL0])
                nc.sync.dma_start(contrib_d[MEM + m * P:MEM + (m + 1) * P, :],
                                  C0[:, m * NL0:(m + 1) * NL0])
            nc.gpsimd.collective_compute(
                "AllGather", ALU.bypass,
                replica_groups=[list(range(NCORES))],
                ins=[contrib_d[:]],
                outs=[gath_d[:]],
            )
            # gath rows: 1024*c + 512*hc + 128*m + p ; cols: slot s.
            # DMA c-major (contiguous 32B runs), then permute columns on an
            # engine to subtree order k = 8*s + c (T2 child column k).
            Hraw = state.tile([P, KT * 64], F32, name="t", tag="Hraw")
            Craw = state.tile([P, KT * 64], F32, name="t", tag="Craw")
            H64 = state.tile([P, KT * 64], F16, name="t", tag="H64")
            C64 = state.tile([P, KT * 64], F32, name="t", tag="C64")
            gv = gath_d[:].rearrange("(c hc m p) s -> hc m p c s",
                                     c=NCORES, hc=2, m=KT)
            for m in range(KT):
                nc.sync.dma_start(
                    Hraw[:, m * 64:(m + 1) * 64].rearrange(
                        "p (c s) -> p c s", c=NCORES), gv[0, m])
                nc.sync.dma_start(
                    Craw[:, m * 64:(m + 1) * 64].rearrange(
                        "p (c s) -> p c s", c=NCORES), gv[1, m])
            perm_in = lambda t: bass.AP(
                tensor=t.tensor, offset=t.offset,
                ap=[t.ap[0], [64, KT], [1, NL0], [NL0, NCORES]])
            perm_out = lambda t: bass.AP(
                tensor=t.tensor, offset=t.offset,
                ap=[t.ap[0], [64, KT], [NL0, NL0], [1, NCORES]])
            nc.gpsimd.tensor_copy(perm_out(H64[:]), perm_in(Hraw[:]))
            nc.gpsimd.tensor_copy(perm_out(C64[:]), perm_in(Craw[:]))

            HT2, CT2 = level_step(16, OFFT2, H64, C64, "T2")
            HT1, CT1 = level_step(4, OFFT1, HT2, CT2, "T1")
            HT0, _ = level_step(1, OFFT0, HT1, CT1, "T0", h_dtype=F32)
            nc.sync.dma_start(out_d[:], HT0[:])

    nc.compile()
    return nc


_NC_CACHE = None


def kernel(inputs, Wx, bx, Ws, bs, Wf, bf, children):
    global LAST_RESULT, _NC_CACHE
    inputs = np.asarray(inputs, np.float32)
    Wx = np.asarray(Wx, np.float32)
    bx = np.asarray(bx, np.float32)
    Ws = np.asarray(Ws, np.float32)
    bs = np.asarray(bs, np.float32)
    Wf = np.asarray(Wf, np.float32)
    bf = np.asarray(bf, np.float32)

    Wx_b = Wx.astype(np.float16)
    Ws_b = Ws.astype(np.float16)
    Wf_b = Wf.astype(np.float16)
    bxT = bx.reshape(16, P).T          # [128, 16] col mc
    bsT = bs.reshape(12, P).T          # [128, 12] col: i0..3 o0..3 u0..3
    bfT = np.ascontiguousarray(bf.reshape(4, P).T)
    # bxc: bx with bs folded into the i/o/u blocks (internal-X bias)
    bxc = bxT.copy()
    bxc[:, 0:4] += bsT[:, 0:4]
    bxc[:, 8:12] += bsT[:, 4:8]
    bxc[:, 12:16] += bsT[:, 8:12]
    bxc = np.ascontiguousarray(bxc)
    # bxs: leaf-gate bias (i,o,u blocks of bx + bs)
    bxs = np.concatenate(
        [bxT[:, 0:4] + bsT[:, 0:4], bxT[:, 8:12] + bsT[:, 4:8],
         bxT[:, 12:16] + bsT[:, 8:12]], axis=1)
    bxs = np.ascontiguousarray(bxs)

    in_maps = []
    for c in range(NCORES):
        heaps = _core_heaps(c)
        valid = heaps >= 0
        M = np.zeros((NX, IN_DIM), np.float32)
        M[valid] = inputs[N - 1 - heaps[valid]]
        xin = np.ascontiguousarray(M.T)
        mrow = valid[NI:].astype(np.float32)
        cmask = np.ascontiguousarray(np.tile(mrow[None, :], (P, 1)))
        in_maps.append({
            "xin": xin.astype(np.float16), "wx": Wx_b, "ws": Ws_b,
            "wf": Wf_b, "bxc": bxc, "bxs": bxs, "bf": bfT, "cmask": cmask,
        })

    if _NC_CACHE is None:
        _NC_CACHE = _build_program()
    nc = _NC_CACHE

    res = run_bass_kernel_spmd(
        nc, in_maps, list(range(NCORES)),
        trace=bool(os.environ.get("BASS_TRACE")),
    )
    LAST_RESULT = res

    out = np.asarray(res.results[0]["out"])  # [128, 4]; h[m*128+p] = out[p, m]
    return np.ascontiguousarray(out.T.reshape(1, MEM))
